# revision 16
# baseline (speedup 1.0000x reference)
"""Trainium2 Bass kernel for BRDFNet (1x1 conv + GGX render), 8-core data parallel.

Sharding: pure data parallel over batch B=8 -> one image per NeuronCore.

Per core (HW = H*W pixels):
- 1x1 conv as feats-stationary fp32 matmuls: lhsT = feats[128c x 128px]
  chunk, rhs = weights [128c x 16ch] -> psum [128px, 16ch], two K-blocks
  accumulated. Output is directly pixel-major (no transpose needed).
- per-pixel GGX math on [128, F]-shaped channel planes (pixel p of chunk g,
  pixel index = g*128 + p).
- All DMAs are contiguous; the host packs wi/wo into that plane layout and
  unpacks the five plane outputs (imgs, normal, albedo, rough, fresnel).
- rsqrt/reciprocal chains use one Newton-Raphson refinement: the ACT sqrt
  table is coarse (~1e-4) for arguments outside [0, ~3] and GGX D amplifies
  NdotH errors ~1e4x at glancing pixels.
"""

import os
import sys

import numpy as np

for _p in ("/opt/trn_rl_repo", "/root/.axon_site/_ro/trn_rl_repo"):
    if os.path.isdir(_p) and _p not in sys.path:
        sys.path.insert(0, _p)

import concourse.bass as bass
import concourse.mybir as mybir
import concourse.tile as tile
from concourse import bacc
from concourse.alu_op_type import AluOpType as ALU
from concourse.bass_utils import run_bass_kernel_spmd

F32 = mybir.dt.float32
AF = mybir.ActivationFunctionType
AX = mybir.AxisListType

PI = 3.141592653589793
LN2 = 0.6931471805599453

B, C, H, W = 8, 256, 256, 256
KB = C // 128  # contraction blocks
NCH = 16       # conv output channels padded 10 -> 16
NCHP = 32      # matmul M padded to a full col-group so psum banks are fully written


def pixel_map(HW):
    """P[p, g] = global pixel index for plane position (p, g)."""
    F = HW // 128
    return (np.arange(F) * 128)[None, :] + np.arange(128)[:, None]  # [128, F]


def build_nc(HW=H * W, NS=8):
    F = HW // 128            # chunks
    FS = F // NS             # chunks per elementwise slice
    SC = F // 16             # superchunks (16 chunks each)
    SCS = SC // NS           # superchunks per slice
    SCB = 2 if SCS % 2 == 0 else 1  # superchunks per feats DMA block
    strip = HW // 4

    nc = bacc.Bacc()
    feats = nc.declare_dram_parameter("feats", [C, HW], F32, isOutput=False)
    wiwo = nc.declare_dram_parameter("wiwo", [128, F, 6], F32, isOutput=False)
    wt = nc.declare_dram_parameter("wt", [128, KB, NCHP], F32, isOutput=False)
    sel = nc.declare_dram_parameter("sel", [128, 64], F32, isOutput=False)
    o_img = nc.declare_dram_parameter("o_img", [128, F * 3], F32, isOutput=True)
    o_nrm = nc.declare_dram_parameter("o_nrm", [128, F * 3], F32, isOutput=True)
    o_alb = nc.declare_dram_parameter("o_alb", [128, F * 3], F32, isOutput=True)
    o_f0 = nc.declare_dram_parameter("o_f0", [128, F * 3], F32, isOutput=True)
    o_ro = nc.declare_dram_parameter("o_ro", [128, F], F32, isOutput=True)

    with tile.TileContext(nc) as tc:
        with (
            tc.tile_pool(name="const", bufs=1) as const,
            tc.tile_pool(name="keep", bufs=1) as keep,
            tc.tile_pool(name="ftile", bufs=4) as fpool,
            tc.tile_pool(name="psc", bufs=4, space="PSUM") as psc,
            tc.tile_pool(name="pst", bufs=4, space="PSUM") as pst,
            tc.tile_pool(name="csb", bufs=3) as csb,
            tc.tile_pool(name="p16", bufs=2) as p16pool,
            tc.tile_pool(name="sl1", bufs=2) as sl1,
            tc.tile_pool(name="sl3", bufs=2) as sl3,
            tc.tile_pool(name="outs", bufs=2) as outs,
        ):
            v = nc.vector
            sc_ = nc.scalar

            # ---- constants / small inputs ----
            wt_sb = const.tile([128, KB, NCHP], F32, tag="wt")
            nc.sync.dma_start(out=wt_sb, in_=wt[:, :, :])
            sel_sb = const.tile([128, 64], F32, tag="sel")
            nc.sync.dma_start(out=sel_sb, in_=sel[:, :])

            wiwo_sb = keep.tile([128, F, 6], F32, tag="wiwo")
            nc.sync.dma_start(out=wiwo_sb, in_=wiwo[:, :, :])

            def rsqrt_refined(y, s, ta, tb):
                """y = 1/max(sqrt(s),1e-12), one NR step. ta/tb scratch."""
                sc_.sqrt(ta, s)
                v.tensor_scalar_max(ta, ta, 1e-12)
                v.reciprocal(y, ta)
                # y' = y * (1.5 - 0.5*s*y^2)
                v.tensor_tensor(out=tb, in0=y, in1=y, op=ALU.mult)
                v.tensor_tensor(out=tb, in0=tb, in1=s, op=ALU.mult)
                v.tensor_scalar(
                    out=tb, in0=tb, scalar1=-0.5, scalar2=1.5,
                    op0=ALU.mult, op1=ALU.add,
                )
                v.tensor_tensor(out=y, in0=y, in1=tb, op=ALU.mult)

            # ---- PRE: quantities depending only on wi/wo (full width) ----
            l3 = keep.tile([128, F, 3], F32, tag="l3")
            v3 = keep.tile([128, F, 3], F32, tag="v3")
            rs_h = keep.tile([128, F], F32, tag="rs_h")
            ePI = keep.tile([128, F], F32, tag="ePI")

            pre3 = keep.tile([128, F, 3], F32, tag="pre3")
            pre1a = keep.tile([128, F], F32, tag="pre1a")
            pre1b = keep.tile([128, F], F32, tag="pre1b")
            pre1c = keep.tile([128, F], F32, tag="pre1c")
            pre1d = keep.tile([128, F], F32, tag="pre1d")

            for (ofs, dst) in ((0, l3), (3, v3)):
                src = wiwo_sb[:, :, ofs:ofs + 3]
                v.tensor_tensor(out=pre3, in0=src, in1=src, op=ALU.mult)
                v.tensor_reduce(out=pre1a, in_=pre3, axis=AX.X, op=ALU.add)
                rsqrt_refined(pre1b, pre1a, pre1c, pre1d)
                v.tensor_tensor(
                    out=dst, in0=src, in1=pre1b.to_broadcast([128, F, 3]),
                    op=ALU.mult,
                )

            v.tensor_tensor(out=pre3, in0=l3, in1=v3, op=ALU.mult)
            ldv = keep.tile([128, F], F32, tag="ldv")
            v.tensor_reduce(out=ldv, in_=pre3, axis=AX.X, op=ALU.add)
            v.tensor_scalar_max(ldv, ldv, -1.0)
            v.tensor_scalar(
                out=pre1a, in0=ldv, scalar1=2.0, scalar2=2.0,
                op0=ALU.mult, op1=ALU.add,
            )
            rsqrt_refined(rs_h, pre1a, pre1c, pre1d)
            vdh = keep.tile([128, F], F32, tag="vdh")
            v.tensor_scalar_add(vdh, ldv, 1.0)
            v.tensor_tensor(out=vdh, in0=vdh, in1=rs_h, op=ALU.mult)
            v.tensor_scalar_max(vdh, vdh, 1e-8)
            v.tensor_scalar(
                out=pre1a, in0=vdh, scalar1=-5.55473 * LN2, scalar2=-6.98316 * LN2,
                op0=ALU.mult, op1=ALU.add,
            )
            v.tensor_tensor(out=pre1a, in0=pre1a, in1=vdh, op=ALU.mult)
            sc_.activation(pre1b, pre1a, AF.Exp)
            v.tensor_scalar_mul(ePI, pre1b, PI)

            # ---- main loop ----
            for sl_i in range(NS):
                sl = slice(sl_i * FS, (sl_i + 1) * FS)
                p16 = p16pool.tile([128, FS * NCH], F32, tag="p16")
                for g in range(FS // 32):
                    pt = psc.tile([128, 32 * NCH], F32, tag="pc")
                    for b_ in range(2):
                        c0 = sl_i * FS + g * 32 + b_ * 16
                        px0 = c0 * 128
                        fh = []
                        for kb in range(KB):
                            t = fpool.tile(
                                [128, 16 * 128], F32, tag=f"fh{kb}",
                                name=f"fh{kb}",
                            )
                            nc.sync.dma_start(
                                out=t,
                                in_=feats[kb * 128:(kb + 1) * 128,
                                          px0:px0 + 16 * 128],
                            )
                            fh.append(t)
                        for j in range(16):
                            jj = b_ * 16 + j
                            for kb in range(KB):
                                nc.tensor.matmul(
                                    pt[:, jj * NCH:(jj + 1) * NCH],
                                    fh[kb][:, j * 128:(j + 1) * 128],
                                    wt_sb[:, kb, 0:NCH],
                                    start=(kb == 0),
                                    stop=(kb == KB - 1),
                                )
                    sc_.copy(p16[:, g * 32 * NCH:(g + 1) * 32 * NCH], pt)

                # ---- elementwise for this slice ----
                p16v = p16.rearrange("p (f c) -> p f c", c=NCH)

                def T1(tag):
                    return sl1.tile([128, FS], F32, tag=tag, name=tag)

                def T3(tag):
                    return sl3.tile([128, FS, 3], F32, tag=tag, name=tag)

                nrm3 = outs.tile([128, FS, 3], F32, tag="nrm3", name="nrm3")
                alb3 = outs.tile([128, FS, 3], F32, tag="alb3", name="alb3")
                f03 = outs.tile([128, FS, 3], F32, tag="f03", name="f03")
                ro = outs.tile([128, FS], F32, tag="ro", name="ro")
                img3 = outs.tile([128, FS, 3], F32, tag="img3", name="img3")

                sc_.activation(nrm3, p16v[:, :, 0:3], AF.Tanh)
                sc_.activation(alb3, p16v[:, :, 3:6], AF.Sigmoid)
                sc_.activation(ro, p16v[:, :, 6], AF.Sigmoid)
                sc_.activation(f03, p16v[:, :, 7:10], AF.Sigmoid)
                v.tensor_scalar_max(ro, ro, 0.001)

                alpha = T1("alpha")
                a2 = T1("a2")
                v.tensor_tensor(out=alpha, in0=ro, in1=ro, op=ALU.mult)
                v.tensor_tensor(out=a2, in0=alpha, in1=alpha, op=ALU.mult)

                p3 = T3("p3")
                t1 = T1("t1")
                t2 = T1("t2")
                t3 = T1("t3")
                t4 = T1("t4")
                v.tensor_tensor(out=p3, in0=nrm3, in1=nrm3, op=ALU.mult)
                v.tensor_reduce(out=t1, in_=p3, axis=AX.X, op=ALU.add)
                rsqrt_refined(t2, t1, t3, t4)
                v.tensor_tensor(
                    out=nrm3, in0=nrm3, in1=t2.to_broadcast([128, FS, 3]),
                    op=ALU.mult,
                )

                nl = T1("nl")
                nv = T1("nv")
                v.tensor_tensor(out=p3, in0=nrm3, in1=l3[:, sl, :], op=ALU.mult)
                v.tensor_reduce(out=nl, in_=p3, axis=AX.X, op=ALU.add)
                v.tensor_tensor(out=p3, in0=nrm3, in1=v3[:, sl, :], op=ALU.mult)
                v.tensor_reduce(out=nv, in_=p3, axis=AX.X, op=ALU.add)

                ndl = T1("ndl")
                ndv = T1("ndv")
                ndh = T1("ndh")
                v.tensor_scalar_max(ndl, nl, 1e-8)
                v.tensor_scalar_max(ndv, nv, 1e-8)
                v.tensor_tensor(out=ndh, in0=nl, in1=nv, op=ALU.add)
                v.tensor_tensor(out=ndh, in0=ndh, in1=rs_h[:, sl], op=ALU.mult)
                v.tensor_scalar_max(ndh, ndh, 1e-8)

                nh2 = T1("nh2")
                a2m1 = T1("a2m1")
                den = T1("den")
                v.tensor_tensor(out=nh2, in0=ndh, in1=ndh, op=ALU.mult)
                v.tensor_scalar_add(a2m1, a2, -1.0)
                v.tensor_tensor(out=den, in0=nh2, in1=a2m1, op=ALU.mult)
                v.tensor_scalar_add(den, den, 1.0)
                v.tensor_tensor(out=den, in0=den, in1=den, op=ALU.mult)
                v.reciprocal(t1, den)
                # refine reciprocal: r' = r*(2 - x*r)
                v.tensor_tensor(out=t3, in0=den, in1=t1, op=ALU.mult)
                v.tensor_scalar(
                    out=t3, in0=t3, scalar1=-1.0, scalar2=2.0,
                    op0=ALU.mult, op1=ALU.add,
                )
                v.tensor_tensor(out=t1, in0=t1, in1=t3, op=ALU.mult)
                draw = T1("draw")
                v.tensor_tensor(out=draw, in0=a2, in1=t1, op=ALU.mult)

                u_ = T1("u_")
                k_ = T1("k_")
                sc_.activation(u_, alpha, AF.Copy, bias=1.0, scale=-0.5)
                sc_.activation(k_, alpha, AF.Copy, bias=0.0, scale=0.5)
                tl_ = T1("tl_")
                tv_ = T1("tv_")
                rl_ = T1("rl_")
                rv_ = T1("rv_")
                v.tensor_tensor(out=tl_, in0=ndl, in1=u_, op=ALU.mult)
                v.tensor_tensor(out=tl_, in0=tl_, in1=k_, op=ALU.add)
                v.reciprocal(rl_, tl_)
                v.tensor_tensor(out=tv_, in0=ndv, in1=u_, op=ALU.mult)
                v.tensor_tensor(out=tv_, in0=tv_, in1=k_, op=ALU.add)
                v.reciprocal(rv_, tv_)
                g_ = T1("g_")
                v.tensor_tensor(out=g_, in0=rl_, in1=rv_, op=ALU.mult)

                dgs = T1("dgs")
                v.tensor_tensor(out=dgs, in0=draw, in1=g_, op=ALU.mult)
                sc_.activation(dgs, dgs, AF.Copy, scale=0.25 / PI)

                omf3 = T3("omf3")
                dif3 = T3("dif3")
                fr3 = T3("fr3")
                sc_.activation(omf3, f03, AF.Copy, scale=-1.0 / PI, bias=1.0 / PI)
                v.tensor_tensor(out=dif3, in0=alb3, in1=omf3, op=ALU.mult)
                v.tensor_tensor(
                    out=fr3, in0=omf3, in1=ePI[:, sl].to_broadcast([128, FS, 3]),
                    op=ALU.mult,
                )
                v.tensor_tensor(out=fr3, in0=fr3, in1=f03, op=ALU.add)
                v.tensor_tensor(
                    out=fr3, in0=fr3, in1=dgs.to_broadcast([128, FS, 3]),
                    op=ALU.mult,
                )
                v.tensor_tensor(out=dif3, in0=dif3, in1=fr3, op=ALU.add)
                v.tensor_tensor(
                    out=dif3, in0=dif3, in1=ndl.to_broadcast([128, FS, 3]),
                    op=ALU.mult,
                )
                sc_.activation(img3, dif3, AF.Relu)

                # ---- outputs for this slice (all contiguous) ----
                s3 = slice(sl_i * FS * 3, (sl_i + 1) * FS * 3)
                nc.sync.dma_start(out=o_img[:, s3], in_=img3)
                nc.sync.dma_start(out=o_nrm[:, s3], in_=nrm3)
                nc.sync.dma_start(out=o_alb[:, s3], in_=alb3)
                nc.sync.dma_start(out=o_f0[:, s3], in_=f03)
                nc.sync.dma_start(out=o_ro[:, sl], in_=ro)

    nc.finalize()
    return nc


_NC_CACHE = {}


def get_nc(**kw):
    key = tuple(sorted(kw.items()))
    if key not in _NC_CACHE:
        _NC_CACHE[key] = build_nc(**kw)
    return _NC_CACHE[key]


def make_sel():
    sel = np.zeros((128, 64), np.float32)
    for r in range(4):
        for i in range(NCH):
            sel[32 * r + i, 16 * r + i] = 1.0
    return sel


def make_in_maps(feats, wi, wo, w_normal, w_albedo, w_rough, w_fresnel):
    w10 = np.concatenate(
        [np.asarray(w_normal), np.asarray(w_albedo),
         np.asarray(w_rough), np.asarray(w_fresnel)], axis=0
    ).astype(np.float32)  # [10, C]
    wt = np.zeros((128, KB, NCHP), np.float32)
    for kb in range(KB):
        wt[:, kb, :10] = w10[:, kb * 128:(kb + 1) * 128].T
    sel = make_sel()
    feats = np.ascontiguousarray(np.asarray(feats, np.float32))
    wi = np.ascontiguousarray(np.asarray(wi, np.float32))
    wo = np.ascontiguousarray(np.asarray(wo, np.float32))
    hw = feats.shape[2] * feats.shape[3]
    P = pixel_map(hw)
    in_maps = []
    for b in range(feats.shape[0]):
        wi_hw = wi[b].reshape(hw, 3)
        wo_hw = wo[b].reshape(hw, 3)
        wiwo = np.concatenate([wi_hw[P], wo_hw[P]], axis=-1)  # [128, F, 6]
        in_maps.append({
            "feats": feats[b].reshape(C, hw),
            "wiwo": np.ascontiguousarray(wiwo),
            "wt": wt,
            "sel": sel,
        })
    return in_maps


def unpack_outputs(res, hw):
    """res: dict of per-core outputs -> (imgs [hw,3], data [hw,12])."""
    P = pixel_map(hw).ravel()
    F = hw // 128
    imgs = np.empty((hw, 3), np.float32)
    data = np.empty((hw, 12), np.float32)
    imgs[P] = np.asarray(res["o_img"]).reshape(128 * F, 3)
    data[P, 0:3] = np.asarray(res["o_nrm"]).reshape(128 * F, 3)
    data[P, 3:6] = np.asarray(res["o_alb"]).reshape(128 * F, 3)
    data[P, 6] = np.asarray(res["o_ro"]).reshape(128 * F)
    data[:, 7] = data[:, 6]
    data[:, 8] = data[:, 6]
    data[P, 9:12] = np.asarray(res["o_f0"]).reshape(128 * F, 3)
    return imgs, data


def kernel(feats, wi, wo, w_normal, w_albedo, w_rough, w_fresnel):
    feats = np.asarray(feats)
    Bb, Cc, Hh, Ww = feats.shape
    hw = Hh * Ww
    in_maps = make_in_maps(feats, wi, wo, w_normal, w_albedo, w_rough, w_fresnel)
    nc = get_nc(HW=hw)
    res = run_bass_kernel_spmd(nc, in_maps, list(range(Bb))).results
    imgs = np.empty((Bb, Hh, Ww, 3), np.float32)
    data = np.empty((Bb, Hh, Ww, 12), np.float32)
    for b in range(Bb):
        im, da = unpack_outputs(res[b], hw)
        imgs[b] = im.reshape(Hh, Ww, 3)
        data[b] = da.reshape(Hh, Ww, 12)
    return imgs, data


# revision 18
# speedup vs baseline: 1.0303x; 1.0303x over previous
"""Trainium2 Bass kernel for BRDFNet (1x1 conv + GGX render), 8-core data parallel.

Sharding: pure data parallel over batch B=8 -> one image per NeuronCore.

Per core (HW = H*W pixels):
- 1x1 conv as feats-stationary fp32 matmuls: lhsT = feats[128c x 128px]
  chunk, rhs = weights [128c x 16ch] -> psum [128px, 16ch], two K-blocks
  accumulated. Output is directly pixel-major (no transpose needed).
- per-pixel GGX math on [128, F]-shaped channel planes (pixel p of chunk g,
  pixel index = g*128 + p).
- All DMAs are contiguous; the host packs wi/wo into that plane layout and
  unpacks the five plane outputs (imgs, normal, albedo, rough, fresnel).
- rsqrt/reciprocal chains use one Newton-Raphson refinement: the ACT sqrt
  table is coarse (~1e-4) for arguments outside [0, ~3] and GGX D amplifies
  NdotH errors ~1e4x at glancing pixels.
"""

import os
import sys

import numpy as np

for _p in ("/opt/trn_rl_repo", "/root/.axon_site/_ro/trn_rl_repo"):
    if os.path.isdir(_p) and _p not in sys.path:
        sys.path.insert(0, _p)

import concourse.bass as bass
import concourse.mybir as mybir
import concourse.tile as tile
from concourse import bacc
from concourse.alu_op_type import AluOpType as ALU
from concourse.bass_utils import run_bass_kernel_spmd

F32 = mybir.dt.float32
AF = mybir.ActivationFunctionType
AX = mybir.AxisListType

PI = 3.141592653589793
LN2 = 0.6931471805599453

B, C, H, W = 8, 256, 256, 256
KB = C // 128  # contraction blocks
NCH = 16       # conv output channels padded 10 -> 16
NCHP = 32      # matmul M padded to a full col-group so psum banks are fully written


def pixel_map(HW):
    """P[p, g] = global pixel index for plane position (p, g)."""
    F = HW // 128
    return (np.arange(F) * 128)[None, :] + np.arange(128)[:, None]  # [128, F]


def build_nc(HW=H * W, NS=4):
    F = HW // 128            # chunks
    FS = F // NS             # chunks per elementwise slice
    SC = F // 16             # superchunks (16 chunks each)
    SCS = SC // NS           # superchunks per slice
    SCB = 2 if SCS % 2 == 0 else 1  # superchunks per feats DMA block
    strip = HW // 4

    nc = bacc.Bacc()
    feats = nc.declare_dram_parameter("feats", [C, HW], F32, isOutput=False)
    wiwo = nc.declare_dram_parameter("wiwo", [128, F, 6], F32, isOutput=False)
    wt = nc.declare_dram_parameter("wt", [128, KB, NCHP], F32, isOutput=False)
    sel = nc.declare_dram_parameter("sel", [128, 64], F32, isOutput=False)
    o_img = nc.declare_dram_parameter("o_img", [128, F * 3], F32, isOutput=True)
    o_nrm = nc.declare_dram_parameter("o_nrm", [128, F * 3], F32, isOutput=True)
    o_alb = nc.declare_dram_parameter("o_alb", [128, F * 3], F32, isOutput=True)
    o_f0 = nc.declare_dram_parameter("o_f0", [128, F * 3], F32, isOutput=True)
    o_ro = nc.declare_dram_parameter("o_ro", [128, F], F32, isOutput=True)

    with tile.TileContext(nc) as tc:
        with (
            tc.tile_pool(name="const", bufs=1) as const,
            tc.tile_pool(name="keep", bufs=1) as keep,
            tc.tile_pool(name="ftile", bufs=3) as fpool,
            tc.tile_pool(name="psc", bufs=4, space="PSUM") as psc,
            tc.tile_pool(name="pst", bufs=4, space="PSUM") as pst,
            tc.tile_pool(name="csb", bufs=3) as csb,
            tc.tile_pool(name="p16", bufs=2) as p16pool,
            tc.tile_pool(name="sl1", bufs=2) as sl1,
            tc.tile_pool(name="sl3", bufs=2) as sl3,
            tc.tile_pool(name="outs", bufs=2) as outs,
        ):
            v = nc.vector
            sc_ = nc.scalar

            # ---- constants / small inputs ----
            wt_sb = const.tile([128, KB, NCHP], F32, tag="wt")
            nc.sync.dma_start(out=wt_sb, in_=wt[:, :, :])
            sel_sb = const.tile([128, 64], F32, tag="sel")
            nc.sync.dma_start(out=sel_sb, in_=sel[:, :])

            wiwo_sb = keep.tile([128, F, 6], F32, tag="wiwo")
            nc.sync.dma_start(out=wiwo_sb, in_=wiwo[:, :, :])

            def rsqrt_refined(y, s, ta, tb):
                """y = 1/max(sqrt(s),1e-12), one NR step. ta/tb scratch."""
                sc_.sqrt(ta, s)
                v.tensor_scalar_max(ta, ta, 1e-12)
                v.reciprocal(y, ta)
                # y' = y * (1.5 - 0.5*s*y^2)
                v.tensor_tensor(out=tb, in0=y, in1=y, op=ALU.mult)
                v.tensor_tensor(out=tb, in0=tb, in1=s, op=ALU.mult)
                v.tensor_scalar(
                    out=tb, in0=tb, scalar1=-0.5, scalar2=1.5,
                    op0=ALU.mult, op1=ALU.add,
                )
                v.tensor_tensor(out=y, in0=y, in1=tb, op=ALU.mult)

            # ---- PRE: quantities depending only on wi/wo (full width) ----
            l3 = keep.tile([128, F, 3], F32, tag="l3")
            v3 = keep.tile([128, F, 3], F32, tag="v3")
            rs_h = keep.tile([128, F], F32, tag="rs_h")
            ePI = keep.tile([128, F], F32, tag="ePI")

            pre3 = keep.tile([128, F, 3], F32, tag="pre3")
            pre1a = keep.tile([128, F], F32, tag="pre1a")
            pre1b = keep.tile([128, F], F32, tag="pre1b")
            pre1c = keep.tile([128, F], F32, tag="pre1c")
            pre1d = keep.tile([128, F], F32, tag="pre1d")

            for (ofs, dst) in ((0, l3), (3, v3)):
                src = wiwo_sb[:, :, ofs:ofs + 3]
                v.tensor_tensor(out=pre3, in0=src, in1=src, op=ALU.mult)
                v.tensor_reduce(out=pre1a, in_=pre3, axis=AX.X, op=ALU.add)
                rsqrt_refined(pre1b, pre1a, pre1c, pre1d)
                v.tensor_tensor(
                    out=dst, in0=src, in1=pre1b.to_broadcast([128, F, 3]),
                    op=ALU.mult,
                )

            v.tensor_tensor(out=pre3, in0=l3, in1=v3, op=ALU.mult)
            ldv = keep.tile([128, F], F32, tag="ldv")
            v.tensor_reduce(out=ldv, in_=pre3, axis=AX.X, op=ALU.add)
            v.tensor_scalar_max(ldv, ldv, -1.0)
            v.tensor_scalar(
                out=pre1a, in0=ldv, scalar1=2.0, scalar2=2.0,
                op0=ALU.mult, op1=ALU.add,
            )
            rsqrt_refined(rs_h, pre1a, pre1c, pre1d)
            vdh = keep.tile([128, F], F32, tag="vdh")
            v.tensor_scalar_add(vdh, ldv, 1.0)
            v.tensor_tensor(out=vdh, in0=vdh, in1=rs_h, op=ALU.mult)
            v.tensor_scalar_max(vdh, vdh, 1e-8)
            v.tensor_scalar(
                out=pre1a, in0=vdh, scalar1=-5.55473 * LN2, scalar2=-6.98316 * LN2,
                op0=ALU.mult, op1=ALU.add,
            )
            v.tensor_tensor(out=pre1a, in0=pre1a, in1=vdh, op=ALU.mult)
            sc_.activation(pre1b, pre1a, AF.Exp)
            v.tensor_scalar_mul(ePI, pre1b, PI)

            # ---- main loop ----
            # uneven slices: big early slices (elementwise efficiency), small
            # final slice (short tail after the last feats DMA)
            if F == 512:
                sizes = [160, 160, 128, 64]
            else:
                sizes = [FS] * NS
            off = 0
            for sl_i, FS in enumerate(sizes):
                sl = slice(off, off + FS)
                p16 = p16pool.tile([128, FS * NCH], F32, tag="p16")
                for g in range(FS // 32):
                    pt = psc.tile([128, 32 * NCH], F32, tag="pc")
                    for b_ in range(2):
                        c0 = off + g * 32 + b_ * 16
                        px0 = c0 * 128
                        fh = []
                        for kb in range(KB):
                            t = fpool.tile(
                                [128, 16 * 128], F32, tag=f"fh{kb}",
                                name=f"fh{kb}",
                            )
                            nc.sync.dma_start(
                                out=t,
                                in_=feats[kb * 128:(kb + 1) * 128,
                                          px0:px0 + 16 * 128],
                            )
                            fh.append(t)
                        for j in range(16):
                            jj = b_ * 16 + j
                            for kb in range(KB):
                                nc.tensor.matmul(
                                    pt[:, jj * NCH:(jj + 1) * NCH],
                                    fh[kb][:, j * 128:(j + 1) * 128],
                                    wt_sb[:, kb, 0:NCH],
                                    start=(kb == 0),
                                    stop=(kb == KB - 1),
                                )
                    sc_.copy(p16[:, g * 32 * NCH:(g + 1) * 32 * NCH], pt)

                # ---- elementwise for this slice ----
                p16v = p16.rearrange("p (f c) -> p f c", c=NCH)

                def T1(tag):
                    return sl1.tile([128, FS], F32, tag=tag, name=tag)

                def T3(tag):
                    return sl3.tile([128, FS, 3], F32, tag=tag, name=tag)

                nrm3 = outs.tile([128, FS, 3], F32, tag="nrm3", name="nrm3")
                alb3 = outs.tile([128, FS, 3], F32, tag="alb3", name="alb3")
                f03 = outs.tile([128, FS, 3], F32, tag="f03", name="f03")
                ro = outs.tile([128, FS], F32, tag="ro", name="ro")
                img3 = outs.tile([128, FS, 3], F32, tag="img3", name="img3")

                sc_.activation(nrm3, p16v[:, :, 0:3], AF.Tanh)
                sc_.activation(alb3, p16v[:, :, 3:6], AF.Sigmoid)
                sc_.activation(ro, p16v[:, :, 6], AF.Sigmoid)
                sc_.activation(f03, p16v[:, :, 7:10], AF.Sigmoid)
                v.tensor_scalar_max(ro, ro, 0.001)

                alpha = T1("alpha")
                a2 = T1("a2")
                v.tensor_tensor(out=alpha, in0=ro, in1=ro, op=ALU.mult)
                v.tensor_tensor(out=a2, in0=alpha, in1=alpha, op=ALU.mult)

                p3 = T3("p3")
                t1 = T1("t1")
                t2 = T1("t2")
                t3 = T1("t3")
                t4 = T1("t4")
                v.tensor_tensor(out=p3, in0=nrm3, in1=nrm3, op=ALU.mult)
                v.tensor_reduce(out=t1, in_=p3, axis=AX.X, op=ALU.add)
                rsqrt_refined(t2, t1, t3, t4)
                v.tensor_tensor(
                    out=nrm3, in0=nrm3, in1=t2.to_broadcast([128, FS, 3]),
                    op=ALU.mult,
                )

                nl = T1("nl")
                nv = T1("nv")
                v.tensor_tensor(out=p3, in0=nrm3, in1=l3[:, sl, :], op=ALU.mult)
                v.tensor_reduce(out=nl, in_=p3, axis=AX.X, op=ALU.add)
                v.tensor_tensor(out=p3, in0=nrm3, in1=v3[:, sl, :], op=ALU.mult)
                v.tensor_reduce(out=nv, in_=p3, axis=AX.X, op=ALU.add)

                ndl = T1("ndl")
                ndv = T1("ndv")
                ndh = T1("ndh")
                v.tensor_scalar_max(ndl, nl, 1e-8)
                v.tensor_scalar_max(ndv, nv, 1e-8)
                v.tensor_tensor(out=ndh, in0=nl, in1=nv, op=ALU.add)
                v.tensor_tensor(out=ndh, in0=ndh, in1=rs_h[:, sl], op=ALU.mult)
                v.tensor_scalar_max(ndh, ndh, 1e-8)

                nh2 = T1("nh2")
                a2m1 = T1("a2m1")
                den = T1("den")
                v.tensor_tensor(out=nh2, in0=ndh, in1=ndh, op=ALU.mult)
                v.tensor_scalar_add(a2m1, a2, -1.0)
                v.tensor_tensor(out=den, in0=nh2, in1=a2m1, op=ALU.mult)
                v.tensor_scalar_add(den, den, 1.0)
                v.tensor_tensor(out=den, in0=den, in1=den, op=ALU.mult)
                v.reciprocal(t1, den)
                # refine reciprocal: r' = r*(2 - x*r)
                v.tensor_tensor(out=t3, in0=den, in1=t1, op=ALU.mult)
                v.tensor_scalar(
                    out=t3, in0=t3, scalar1=-1.0, scalar2=2.0,
                    op0=ALU.mult, op1=ALU.add,
                )
                v.tensor_tensor(out=t1, in0=t1, in1=t3, op=ALU.mult)
                draw = T1("draw")
                v.tensor_tensor(out=draw, in0=a2, in1=t1, op=ALU.mult)

                u_ = T1("u_")
                k_ = T1("k_")
                sc_.activation(u_, alpha, AF.Copy, bias=1.0, scale=-0.5)
                sc_.activation(k_, alpha, AF.Copy, bias=0.0, scale=0.5)
                tl_ = T1("tl_")
                tv_ = T1("tv_")
                rl_ = T1("rl_")
                rv_ = T1("rv_")
                v.tensor_tensor(out=tl_, in0=ndl, in1=u_, op=ALU.mult)
                v.tensor_tensor(out=tl_, in0=tl_, in1=k_, op=ALU.add)
                v.reciprocal(rl_, tl_)
                v.tensor_tensor(out=tv_, in0=ndv, in1=u_, op=ALU.mult)
                v.tensor_tensor(out=tv_, in0=tv_, in1=k_, op=ALU.add)
                v.reciprocal(rv_, tv_)
                g_ = T1("g_")
                v.tensor_tensor(out=g_, in0=rl_, in1=rv_, op=ALU.mult)

                dgs = T1("dgs")
                v.tensor_tensor(out=dgs, in0=draw, in1=g_, op=ALU.mult)
                sc_.activation(dgs, dgs, AF.Copy, scale=0.25 / PI)

                omf3 = T3("omf3")
                dif3 = T3("dif3")
                fr3 = T3("fr3")
                sc_.activation(omf3, f03, AF.Copy, scale=-1.0 / PI, bias=1.0 / PI)
                v.tensor_tensor(out=dif3, in0=alb3, in1=omf3, op=ALU.mult)
                v.tensor_tensor(
                    out=fr3, in0=omf3, in1=ePI[:, sl].to_broadcast([128, FS, 3]),
                    op=ALU.mult,
                )
                v.tensor_tensor(out=fr3, in0=fr3, in1=f03, op=ALU.add)
                v.tensor_tensor(
                    out=fr3, in0=fr3, in1=dgs.to_broadcast([128, FS, 3]),
                    op=ALU.mult,
                )
                v.tensor_tensor(out=dif3, in0=dif3, in1=fr3, op=ALU.add)
                v.tensor_tensor(
                    out=dif3, in0=dif3, in1=ndl.to_broadcast([128, FS, 3]),
                    op=ALU.mult,
                )
                sc_.activation(img3, dif3, AF.Relu)

                # ---- outputs for this slice (all contiguous) ----
                s3 = slice(off * 3, (off + FS) * 3)
                nc.sync.dma_start(out=o_img[:, s3], in_=img3)
                nc.sync.dma_start(out=o_nrm[:, s3], in_=nrm3)
                nc.sync.dma_start(out=o_alb[:, s3], in_=alb3)
                nc.sync.dma_start(out=o_f0[:, s3], in_=f03)
                nc.sync.dma_start(out=o_ro[:, sl], in_=ro)
                off += FS

    nc.finalize()
    return nc


_NC_CACHE = {}


def get_nc(**kw):
    key = tuple(sorted(kw.items()))
    if key not in _NC_CACHE:
        _NC_CACHE[key] = build_nc(**kw)
    return _NC_CACHE[key]


def make_sel():
    sel = np.zeros((128, 64), np.float32)
    for r in range(4):
        for i in range(NCH):
            sel[32 * r + i, 16 * r + i] = 1.0
    return sel


def make_in_maps(feats, wi, wo, w_normal, w_albedo, w_rough, w_fresnel):
    w10 = np.concatenate(
        [np.asarray(w_normal), np.asarray(w_albedo),
         np.asarray(w_rough), np.asarray(w_fresnel)], axis=0
    ).astype(np.float32)  # [10, C]
    wt = np.zeros((128, KB, NCHP), np.float32)
    for kb in range(KB):
        wt[:, kb, :10] = w10[:, kb * 128:(kb + 1) * 128].T
    sel = make_sel()
    feats = np.ascontiguousarray(np.asarray(feats, np.float32))
    wi = np.ascontiguousarray(np.asarray(wi, np.float32))
    wo = np.ascontiguousarray(np.asarray(wo, np.float32))
    hw = feats.shape[2] * feats.shape[3]
    P = pixel_map(hw)
    in_maps = []
    for b in range(feats.shape[0]):
        wi_hw = wi[b].reshape(hw, 3)
        wo_hw = wo[b].reshape(hw, 3)
        wiwo = np.concatenate([wi_hw[P], wo_hw[P]], axis=-1)  # [128, F, 6]
        in_maps.append({
            "feats": feats[b].reshape(C, hw),
            "wiwo": np.ascontiguousarray(wiwo),
            "wt": wt,
            "sel": sel,
        })
    return in_maps


def unpack_outputs(res, hw):
    """res: dict of per-core outputs -> (imgs [hw,3], data [hw,12])."""
    P = pixel_map(hw).ravel()
    F = hw // 128
    imgs = np.empty((hw, 3), np.float32)
    data = np.empty((hw, 12), np.float32)
    imgs[P] = np.asarray(res["o_img"]).reshape(128 * F, 3)
    data[P, 0:3] = np.asarray(res["o_nrm"]).reshape(128 * F, 3)
    data[P, 3:6] = np.asarray(res["o_alb"]).reshape(128 * F, 3)
    data[P, 6] = np.asarray(res["o_ro"]).reshape(128 * F)
    data[:, 7] = data[:, 6]
    data[:, 8] = data[:, 6]
    data[P, 9:12] = np.asarray(res["o_f0"]).reshape(128 * F, 3)
    return imgs, data


def kernel(feats, wi, wo, w_normal, w_albedo, w_rough, w_fresnel):
    feats = np.asarray(feats)
    Bb, Cc, Hh, Ww = feats.shape
    hw = Hh * Ww
    in_maps = make_in_maps(feats, wi, wo, w_normal, w_albedo, w_rough, w_fresnel)
    nc = get_nc(HW=hw)
    res = run_bass_kernel_spmd(nc, in_maps, list(range(Bb))).results
    imgs = np.empty((Bb, Hh, Ww, 3), np.float32)
    data = np.empty((Bb, Hh, Ww, 12), np.float32)
    for b in range(Bb):
        im, da = unpack_outputs(res[b], hw)
        imgs[b] = im.reshape(Hh, Ww, 3)
        data[b] = da.reshape(Hh, Ww, 12)
    return imgs, data
